# revision 1
# baseline (speedup 1.0000x reference)
"""AttentionSinkPrefill Trainium2 kernel (8 NeuronCores, sequence-parallel).

Module:   Y = AttnSinkPrefill(X) with sink=4, window=256, causal GQA
          (16 q heads, 4 kv heads, head_dim 64, d_model 1024, B=2, T=2048).

Sharding: sequence-parallel over T.  Core c handles queries
          [256c, 256c+256) for both batches.  Because attention is
          sink+window sparse, each core only needs X rows
          [256c-256, 256c+256) (zero-padded at the left boundary) plus the
          4 sink rows, and computes its o_proj output rows completely --
          no collective and no host-side reduction, outputs concatenate.

Per-core pipeline (single uniform program; per-core variation enters only
through the input data -- padded X slices and masks precomputed on host):
  1. DMA X window tiles, transpose on PE -> X^T  [d_model, keys] layout
  2. projections (PE):  Q^T = Wq^T X^T (per head), K^T, V (keys-major)
  3. per (head, batch): S^T = K^T^T.T @ Q^T -> exp (ACT) -> multiplicative
     mask (DVE) -> Y^T = V_aug^T P^T where V_aug has a ones column so the
     softmax denominator falls out of the same matmuls -> normalize
  4. O = Y_flat @ Wo  (PE), DMA out

Host-side tricks: Wq is pre-scaled by 1/sqrt(64) and its columns permuted
(Wo rows likewise) so every q head lands at the same SBUF partition base
as its kv head's K^T rows (matmul requires equal base partitions).
"""

import os
import sys
from contextlib import ExitStack

import numpy as np

sys.path.insert(0, "/opt/trn_rl_repo")

import concourse.bass as bass
import concourse.bacc as bacc
import concourse.mybir as mybir
import concourse.tile as tile
from concourse.bass_utils import run_bass_kernel_spmd

# ---------------------------------------------------------------- constants
D = 1024          # d_model
NH = 16           # q heads
NKV = 4           # kv heads
HD = 64           # head dim
SINK = 4          # attention sink width
WIN = 256         # sliding window
B = 2
T = 2048
NCORES = 8
QB = T // NCORES  # queries per core = 256
KW = 2 * QB       # window key rows per core = 512

F32 = mybir.dt.float32
# matmul compute dtype: float32r = fp32 data, reduced-precision fast matmul
# (1 cycle/row at N>=256 vs 4 for plain fp32).  Flip to float32 if accuracy
# on hardware turns out insufficient.
MM_DT = (mybir.dt.bfloat16 if os.environ.get("K_DT", "f32r") == "bf16"
         else mybir.dt.float32r)
MM = MM_DT
NP_MM = "bfloat16" if MM_DT == mybir.dt.bfloat16 else "float32"
FR = mybir.dt.float32r  # denominator-broadcast chain stays fp32r
USE_FAST_RECIP = os.environ.get("K_FAST_RECIP", "1") == "1"
MERGED_Q = os.environ.get("K_MERGED_Q", "1") == "1"
SINK_IN_YS = os.environ.get("K_SINK_IN_YS", "1") == "1"
NEW_RECIP = os.environ.get("K_NEW_RECIP", "1") == "1"
PBUFS = int(os.environ.get("K_PBUFS", "2"))
SBUFS = int(os.environ.get("K_SBUFS", "3"))

AF = mybir.ActivationFunctionType

# head order placing each q head at partition base (kv_head%2)*64, paired
# (h, h+4) per 128-feature tile; Wq columns / Wo rows are permuted to match.
HEAD_ORDER = [0, 4, 1, 5, 2, 6, 3, 7, 8, 12, 9, 13, 10, 14, 11, 15]
HEAD_POS = {h: i for i, h in enumerate(HEAD_ORDER)}


# ================================================================ program
def build_nc():
    nc = bacc.Bacc()

    xw_d = nc.dram_tensor("Xw", [B, KW, D], F32, kind="ExternalInput")
    xs_d = nc.dram_tensor("Xs", [B, SINK, D], F32, kind="ExternalInput")
    wq_d = nc.dram_tensor("Wq", [D, NH * HD], MM, kind="ExternalInput")
    wk_d = nc.dram_tensor("Wk", [D, NKV * HD], MM, kind="ExternalInput")
    wv_d = nc.dram_tensor("Wv", [D, NKV * HD], MM, kind="ExternalInput")
    wo_d = nc.dram_tensor("Wo", [NH * HD, D], MM, kind="ExternalInput")
    mtw_d = nc.dram_tensor("MTw", [128, 4 * QB], MM, kind="ExternalInput")
    mts_d = nc.dram_tensor("MTs", [SINK, QB], MM, kind="ExternalInput")
    zer_d = nc.dram_tensor("ZER", [128, 128], MM, kind="ExternalInput")
    one_d = nc.dram_tensor("ONE", [128, 64], MM, kind="ExternalInput")
    oner_d = nc.dram_tensor("ONER", [128, 64], FR, kind="ExternalInput")
    out_d = nc.dram_tensor("out", [B, QB, D], F32, kind="ExternalOutput")

    ident_d = nc.inline_tensor(np.eye(128, dtype=np.float32), name="ident")

    KCOL = KW + SINK  # 516 key columns per batch in X^T layout

    with nc.allow_low_precision(reason="f32r matmul operands"), \
            tile.TileContext(nc) as tc, ExitStack() as ctx:
        consts = ctx.enter_context(tc.tile_pool(name="consts", bufs=1))
        wpool = ctx.enter_context(tc.tile_pool(name="wpool", bufs=1))
        # big streaming pool: X window tiles (stage 1) then Wo (stage 4)
        big = ctx.enter_context(tc.tile_pool(name="big", bufs=1))
        xtp = ctx.enter_context(tc.tile_pool(name="xt", bufs=1))
        qkv = ctx.enter_context(tc.tile_pool(name="qkv", bufs=1))
        ppool = ctx.enter_context(tc.tile_pool(name="pp", bufs=PBUFS))
        ypool = ctx.enter_context(tc.tile_pool(name="yp", bufs=1))
        spool = ctx.enter_context(tc.tile_pool(name="sp", bufs=SBUFS))
        opool = ctx.enter_context(tc.tile_pool(name="op", bufs=2))
        psA = ctx.enter_context(tc.tile_pool(name="psA", bufs=2, space="PSUM"))
        psS = ctx.enter_context(tc.tile_pool(name="psS", bufs=2, space="PSUM"))

        ident = consts.tile([128, 128], F32, tag="ident")
        nc.sync.dma_start(ident[:], ident_d[:])
        mtw = consts.tile([128, 4 * QB], MM, tag="mtw")
        nc.sync.dma_start(mtw[:], mtw_d[:])
        mts = consts.tile([SINK, QB], MM, tag="mts")
        nc.sync.dma_start(mts[:], mts_d[:])

        wq = []
        wk = []
        wv = []
        for d in range(8):
            t = wpool.tile([128, NH * HD], MM, tag=f"wq{d}", name=f"wq{d}")
            nc.sync.dma_start(t[:], wq_d[d * 128:(d + 1) * 128, :])
            wq.append(t)
            t = wpool.tile([128, NKV * HD], MM, tag=f"wk{d}", name=f"wk{d}")
            nc.sync.dma_start(t[:], wk_d[d * 128:(d + 1) * 128, :])
            wk.append(t)
            t = wpool.tile([128, NKV * HD], MM, tag=f"wv{d}", name=f"wv{d}")
            nc.sync.dma_start(t[:], wv_d[d * 128:(d + 1) * 128, :])
            wv.append(t)

        # persistent per-core tensors
        xt = [xtp.tile([128, B * KCOL], MM, tag=f"xt{d}", name=f"xt{d}")
              for d in range(8)]
        qt = [qkv.tile([128, B * QB], MM, tag=f"qt{m}", name=f"qt{m}")
              for m in range(8)]
        kt = [qkv.tile([128, B * KW], MM, tag=f"kt{m}", name=f"kt{m}")
              for m in range(2)]
        # zero-padded sink K^T tiles: [feat 128, key 0:4 real | 4:128 zero]
        ktp = {}
        for m in range(2):
            for b in range(B):
                tl = qkv.tile([128, 128], MM, tag=f"ktp{m}{b}", name=f"ktp{m}{b}")
                nc.sync.dma_start(tl[:], zer_d[:])
                ktp[(m, b)] = tl
        # V in keys-major layout with a ones column per kv head (denominator)
        vt = {}
        for tki in range(4):
            for b in range(B):
                tl = qkv.tile([128, NKV * (HD + 1)], MM,
                              tag=f"vt{tki}{b}", name=f"vt{tki}{b}")
                nc.sync.dma_start(tl[:, 64:NKV * 65:65], one_d[:, 0:NKV])
                vt[(tki, b)] = tl
        vs = {}
        for b in range(B):
            tl = qkv.tile([SINK, NKV * (HD + 1)], MM, tag=f"vs{b}", name=f"vs{b}")
            nc.sync.dma_start(tl[0:SINK, 64:NKV * 65:65], one_d[0:SINK, 0:NKV])
            vs[b] = tl
        yt = [ypool.tile([128, B * QB], MM, tag=f"yt{m}", name=f"yt{m}")
              for m in range(8)]
        # ones column used to broadcast the softmax denominator across
        # partitions via a K=1 matmul (row 64 matches ys's denominator row)
        ones = consts.tile([128, 64], FR, tag="ones")
        nc.sync.dma_start(ones[:], oner_d[:])

        # ---------------- stage 1+2 per batch: X^T, then Q/K/V projections
        for b in range(B):
            xws = []
            for tki in range(4):
                xwt = big.tile([128, D], F32, tag=f"bg{tki}", name=f"xw{tki}_{b}")
                nc.sync.dma_start(xwt[:], xw_d[b, tki * 128:(tki + 1) * 128, :])
                xws.append(xwt)
            xsk = big.tile([128, D], F32, tag="bg4", name=f"xs_{b}")
            nc.sync.dma_start(xsk[0:SINK, :], xs_d[b])

            for d in range(8):
                ps = psA.tile([128, 512], F32, tag="ys", name=f"trps{b}{d}")
                for tki in range(4):
                    nc.tensor.transpose(
                        ps[:, tki * 128:(tki + 1) * 128],
                        xws[tki][:, d * 128:(d + 1) * 128],
                        ident[:],
                    )
                nc.scalar.copy(xt[d][:, b * KCOL:b * KCOL + KW], ps[:])
                ps2 = psA.tile([128, 512], F32, tag="ys", name=f"trps2{b}{d}")
                nc.tensor.transpose(
                    ps2[:, 0:SINK],
                    xsk[0:SINK, d * 128:(d + 1) * 128],
                    ident[0:SINK, 0:SINK],
                )
                nc.scalar.copy(
                    xt[d][:, b * KCOL + KW:b * KCOL + KCOL], ps2[:, 0:SINK]
                )

            # Q^T moved out of the per-batch loop (runs once, both batches)

            # K^T: window part and sink part
            for m in range(2):
                ps = psA.tile([128, 512], F32, tag="ys", name=f"kps{b}{m}")
                for d in range(8):
                    nc.tensor.matmul(
                        ps[:],
                        wk[d][:, m * 128:(m + 1) * 128],
                        xt[d][:, b * KCOL:b * KCOL + KW],
                        start=(d == 0), stop=(d == 7),
                    )
                nc.vector.tensor_copy(kt[m][:, b * KW:(b + 1) * KW], ps[:])
                ps2 = psA.tile([128, 512], F32, tag="ys", name=f"ksps{b}{m}")
                for d in range(8):
                    nc.tensor.matmul(
                        ps2[:, 0:SINK],
                        wk[d][:, m * 128:(m + 1) * 128],
                        xt[d][:, b * KCOL + KW:b * KCOL + KCOL],
                        start=(d == 0), stop=(d == 7),
                    )
                nc.vector.tensor_copy(ktp[(m, b)][:, 0:SINK], ps2[:, 0:SINK])

            # V (keys-major) + sink V
            for tki in range(4):
                ps = psA.tile([128, 512], F32, tag="ys", name=f"vps{b}{tki}")
                for d in range(8):
                    nc.tensor.matmul(
                        ps[:, 0:NKV * HD],
                        xt[d][:, b * KCOL + tki * 128:b * KCOL + (tki + 1) * 128],
                        wv[d][:],
                        start=(d == 0), stop=(d == 7),
                    )
                for g in range(NKV):
                    nc.vector.tensor_copy(
                        vt[(tki, b)][:, g * 65:g * 65 + HD],
                        ps[:, g * HD:(g + 1) * HD],
                    )
            ps = psA.tile([128, 512], F32, tag="ys", name=f"vsps{b}")
            for d in range(8):
                nc.tensor.matmul(
                    ps[0:SINK, 0:NKV * HD],
                    xt[d][:, b * KCOL + KW:b * KCOL + KCOL],
                    wv[d][:],
                    start=(d == 0), stop=(d == 7),
                )
            for g in range(NKV):
                nc.vector.tensor_copy(
                    vs[b][0:SINK, g * 65:g * 65 + HD],
                    ps[0:SINK, g * HD:(g + 1) * HD],
                )

        # Q^T: both batches in one N=512 matmul per (m, d); query columns
        # of X^T sit at offset KW-QB within each batch's KCOL-wide block
        for m in range(8):
            if MERGED_Q:
                ps = psA.tile([128, 512], F32, tag="ys", name=f"qps{m}")
                for d in range(8):
                    rhs = xt[d][:].rearrange(
                        "p (b c) -> p b c", b=B
                    )[:, :, KW - QB:KW]
                    nc.tensor.matmul(
                        ps[:],
                        wq[d][:, m * 128:(m + 1) * 128],
                        rhs,
                        start=(d == 0), stop=(d == 7),
                    )
                nc.vector.tensor_copy(qt[m][:], ps[:])
            else:
                for b2 in range(B):
                    ps = psA.tile([128, 512], F32, tag="ys", name=f"qps{m}{b2}")
                    for d in range(8):
                        nc.tensor.matmul(
                            ps[:, 0:QB],
                            wq[d][:, m * 128:(m + 1) * 128],
                            xt[d][:, b2 * KCOL + KW - QB:b2 * KCOL + KW],
                            start=(d == 0), stop=(d == 7),
                        )
                    nc.vector.tensor_copy(
                        qt[m][:, b2 * QB:(b2 + 1) * QB], ps[:, 0:QB])

        # ---------------- stage 3: attention per (batch, head)
        for b in range(B):
            for h in range(NH):
                g = h // 4           # kv head
                mk = g // 2          # K^T tile index
                kb = (g % 2) * 64    # partition base of this kv head's K^T/Q^T
                pos = HEAD_POS[h]
                mq = pos // 2        # Q^T tile index (post-permutation)

                qrhs = qt[mq][kb:kb + 64, b * QB:(b + 1) * QB]

                swid = 4 * QB if SINK_IN_YS else 5 * QB
                sp = psS.tile([128, swid], F32, tag="s", name=f"s{b}{h}")
                ys = psA.tile([128, 512], F32, tag="ys", name=f"ys{b}{h}")
                for tki in range(4):
                    nc.tensor.matmul(
                        sp[:, tki * QB:(tki + 1) * QB],
                        kt[mk][kb:kb + 64, b * KW + tki * 128:b * KW + (tki + 1) * 128],
                        qrhs,
                        start=True, stop=True,
                    )
                sink_dst = ys[:, QB:2 * QB] if SINK_IN_YS else sp[:, 4 * QB:5 * QB]
                nc.tensor.matmul(
                    sink_dst,
                    ktp[(mk, b)][kb:kb + 64, :],
                    qrhs,
                    start=True, stop=True,
                )

                p = ppool.tile([128, 5 * QB], MM, tag="p", name=f"p{b}{h}")
                if SINK_IN_YS:
                    nc.scalar.activation(p[:, 0:4 * QB], sp[:], AF.Exp)
                    nc.scalar.activation(p[:, 4 * QB:5 * QB], ys[:, QB:2 * QB], AF.Exp)
                else:
                    nc.scalar.activation(p[:], sp[:], AF.Exp)
                nc.vector.tensor_mul(p[:, 0:4 * QB], p[:, 0:4 * QB], mtw[:])
                nc.vector.tensor_mul(
                    p[0:SINK, 4 * QB:5 * QB], p[0:SINK, 4 * QB:5 * QB], mts[:]
                )

                for tki in range(4):
                    nc.tensor.matmul(
                        ys[0:HD + 1, 0:QB],
                        vt[(tki, b)][:, g * 65:g * 65 + 65],
                        p[:, tki * QB:(tki + 1) * QB],
                        start=(tki == 0), stop=False,
                    )
                nc.tensor.matmul(
                    ys[0:HD + 1, 0:QB],
                    vs[b][0:SINK, g * 65:g * 65 + 65],
                    p[0:SINK, 4 * QB:5 * QB],
                    start=False, stop=True,
                )

                # normalize: row HD of ys is the softmax denominator
                rbp = psA.tile([64, QB], F32,
                               tag="rb" if SINK_IN_YS else "ys",
                               name=f"rbp{b}{h}")
                rb = spool.tile([64, QB], F32, tag="rb", name=f"rb{b}{h}")
                dn = spool.tile([HD + 1, QB], FR, tag="rc", name=f"rc{b}{h}")
                if NEW_RECIP:
                    # copy denom to SBUF, broadcast via K=1 matmul, then
                    # one reciprocal over the broadcast block
                    nc.scalar.copy(dn[HD:HD + 1, :], ys[HD:HD + 1, 0:QB])
                    nc.tensor.matmul(
                        rbp[:], ones[HD:HD + 1, :], dn[HD:HD + 1, :],
                        start=True, stop=True,
                    )
                    if USE_FAST_RECIP:
                        nc.vector.reciprocal_approx_fast(rb[:], rbp[:])
                    else:
                        nc.vector.reciprocal(rb[:], rbp[:])
                else:
                    # iteration-1 proven chain: reciprocal first, then
                    # broadcast, then ACT copy to SBUF
                    nc.vector.reciprocal(dn[HD:HD + 1, :], ys[HD:HD + 1, 0:QB])
                    nc.tensor.matmul(
                        rbp[:], ones[HD:HD + 1, :], dn[HD:HD + 1, :],
                        start=True, stop=True,
                    )
                    nc.scalar.copy(rb[:], rbp[:])
                if kb == 0:
                    nc.vector.tensor_mul(
                        yt[mq][0:64, b * QB:(b + 1) * QB], ys[0:HD, 0:QB], rb[:]
                    )
                else:
                    stg = spool.tile([64, QB], MM, tag="stg", name=f"stg{b}{h}")
                    nc.vector.tensor_mul(stg[:], ys[0:HD, 0:QB], rb[:])
                    nc.sync.dma_start(
                        yt[mq][kb:kb + 64, b * QB:(b + 1) * QB], stg[:]
                    )

        # ---------------- stage 4: O projection
        wo = []
        for m in range(8):
            t = big.tile([128, D], MM, tag=f"bg{m}", name=f"wo{m}")
            nc.sync.dma_start(t[:], wo_d[m * 128:(m + 1) * 128, :])
            wo.append(t)
        for b in range(B):
            for mq2 in range(2):
                for nk in range(2):
                    po = psA.tile([128, 512], F32, tag="ys", name=f"po{b}{mq2}{nk}")
                    for m in range(8):
                        nc.tensor.matmul(
                            po[:],
                            yt[m][:, b * QB + mq2 * 128:b * QB + (mq2 + 1) * 128],
                            wo[m][:, nk * 512:(nk + 1) * 512],
                            start=(m == 0), stop=(m == 7),
                        )
                    ost = opool.tile([128, 512], F32, tag="ost", name=f"o{b}{mq2}{nk}")
                    nc.scalar.copy(ost[:], po[:])
                    nc.sync.dma_start(
                        out_d[b, mq2 * 128:(mq2 + 1) * 128, nk * 512:(nk + 1) * 512],
                        ost[:],
                    )

    nc.compile()
    return nc


# ================================================================ host side
def host_prep(X, Wq, Wk, Wv, Wo):
    """Returns in_maps (list of per-core dicts of numpy arrays)."""
    X = np.asarray(X, dtype=np.float32)
    Wq = np.asarray(Wq, dtype=np.float32)
    Wk = np.asarray(Wk, dtype=np.float32)
    Wv = np.asarray(Wv, dtype=np.float32)
    Wo = np.asarray(Wo, dtype=np.float32)

    flat_perm = np.concatenate(
        [np.arange(h * HD, (h + 1) * HD) for h in HEAD_ORDER]
    )
    wq_p = np.ascontiguousarray(Wq[:, flat_perm]) * np.float32(1.0 / np.sqrt(HD))
    wo_p = np.ascontiguousarray(Wo[flat_perm, :])

    tt = np.arange(T)
    i = tt[:, None]
    j = tt[None, :]
    m_full = (j <= i) & ((j < SINK) | (j >= np.maximum(i - WIN + 1, 0)))
    m_full = m_full.astype(np.float32)

    xs = np.ascontiguousarray(X[:, 0:SINK, :])

    in_maps = []
    for c in range(NCORES):
        qs = c * QB
        ks = qs - QB  # window starts one query-block earlier (512 rows)

        xw = np.zeros((B, KW, D), dtype=np.float32)
        lo = max(ks, 0)
        xw[:, lo - ks:, :] = X[:, lo:ks + KW, :]

        # window mask, transposed: [key 512, query 256] -> [128, 4*256]
        mtw = np.zeros((KW, QB), dtype=np.float32)
        jg = ks + np.arange(KW)
        valid = jg >= 0
        mtw[valid, :] = m_full[qs:qs + QB, jg[valid]].T
        mtw_sb = np.ascontiguousarray(
            mtw.reshape(4, 128, QB).transpose(1, 0, 2).reshape(128, 4 * QB)
        )

        # sink mask [4, 256]; zero where the window tiles already cover col j
        mts = np.zeros((SINK, QB), dtype=np.float32)
        for jj in range(SINK):
            if not (ks <= jj < ks + KW):
                mts[jj, :] = m_full[qs:qs + QB, jj]

        import ml_dtypes
        np_mm = np.dtype(ml_dtypes.bfloat16) if NP_MM == "bfloat16" else np.float32
        in_maps.append({
            "ZER": np.zeros((128, 128), dtype=np_mm),
            "ONE": np.ones((128, 64), dtype=np_mm),
            "ONER": np.ones((128, 64), dtype=np.float32),
            "Xw": xw,
            "Xs": xs,
            "Wq": wq_p.astype(np_mm),
            "Wk": Wk.astype(np_mm),
            "Wv": Wv.astype(np_mm),
            "Wo": wo_p.astype(np_mm),
            "MTw": mtw_sb.astype(np_mm),
            "MTs": mts.astype(np_mm),
        })
    return in_maps


_NC_CACHE = {}


def get_nc():
    if "nc" not in _NC_CACHE:
        _NC_CACHE["nc"] = build_nc()
    return _NC_CACHE["nc"]


def kernel(X, Wq, Wk, Wv, Wo):
    in_maps = host_prep(X, Wq, Wk, Wv, Wo)
    nc = get_nc()
    res = run_bass_kernel_spmd(nc, in_maps, list(range(NCORES)))
    out = np.empty((B, T, D), dtype=np.float32)
    for c in range(NCORES):
        out[:, c * QB:(c + 1) * QB, :] = res.results[c]["out"]
    return out

